# revision 19
# baseline (speedup 1.0000x reference)
"""Bahdanau-attention kernel for Trainium2 (8 NeuronCores, batch-sharded).

Reference computation (per batch b):
  att1 = enc[b] @ We + be                    # [L, A]
  att2 = dec[b] @ Wd + bd                    # [A]
  att  = tanh(att1 + att2) @ wf (+ bf)       # [L]   (bf cancels in softmax)
  att  = where(mask==0, -1e9, att)
  alpha = softmax(att)                       # [L]
  awe  = alpha @ enc[b]                      # [E]
Outputs: (awe [B,E] f32, alpha [B,L] f32)

Per-core strategy (32 local batches, processed in pairs):
  - DMA enc natural [l, e] tiles; PE-transpose to X^T [e, l] pair tiles.
  - att1^T [a, l] via PE: stationary = We tile (natural layout), moving = X^T
    pair tile (N=392 >= 256 so float32r streams at 1 cycle/row, full fp32 bits).
  - tanh on ACT reading PSUM, bias = column of att2^T (bd+be folded in).
  - scores via PE (wf^T stationary, M=1) -> [1, 392] at partition 0;
    additive-masked softmax there.
  - awe via DVE scalar_tensor_tensor with fused accum (sum over l), weighting
    X^T tiles with a PE-broadcast alpha.
"""

import sys

for _p in ("/opt/trn_rl_repo",):
    if _p not in sys.path:
        sys.path.insert(0, _p)

from contextlib import ExitStack

import numpy as np

import concourse.bass as bass
import concourse.tile as tile
from concourse import bacc
from concourse import mybir
from concourse.masks import make_identity

B, L, E, D, A = 256, 196, 2048, 1024, 512
NCORES = 8
BL = B // NCORES          # 32 batches per core
NPAIR = BL // 2           # 16 pairs
ET, AT, DT = E // 128, A // 128, D // 128   # 16, 4, 8
L0, L1 = 128, L - 128     # 128, 68
W2 = 2 * L                # 392 (pair-packed free width)

F32 = mybir.dt.float32
F32R = mybir.dt.float32r
I32 = mybir.dt.int32

# Streaming dtype for the big matmuls: "f32r" (full fp32 bits, fast streaming
# mode) or "f32" (4x slower). Flip if f32r misbehaves on HW.
MM_MODE = "f32r"
MMDT = F32R if MM_MODE == "f32r" else F32


def build_kernel(ctx: ExitStack, tc: tile.TileContext, aps):
    nc = tc.nc
    enc, dec, msk = aps["enc"], aps["dec"], aps["msk"]
    w_we, w_be, w_wd, w_bd, w_wf = aps["We"], aps["be"], aps["Wd"], aps["bd"], aps["wf"]
    awe_o, alp_o = aps["awe"], aps["alp"]

    const = ctx.enter_context(tc.tile_pool(name="const", bufs=1))
    # PSUM: 8 banks total = 2 + 3 + 1 + 2
    psT = ctx.enter_context(tc.tile_pool(name="psT", bufs=2, space="PSUM"))
    psA = ctx.enter_context(tc.tile_pool(name="psA", bufs=3, space="PSUM"))
    psS = ctx.enter_context(tc.tile_pool(name="psS", bufs=1, space="PSUM"))
    psB = ctx.enter_context(tc.tile_pool(name="psB", bufs=2, space="PSUM"))

    # ---------------- constants / preamble ----------------
    ident = const.tile([128, 128], F32)
    make_identity(nc, ident)
    ident_r = const.tile([128, 128], MMDT)
    nc.vector.tensor_copy(out=ident_r, in_=ident)

    ones_st = const.tile([1, 128], F32)
    nc.vector.memset(ones_st, 1.0)
    ones_row = const.tile([1, 128], MMDT)
    nc.vector.tensor_copy(out=ones_row, in_=ones_st)

    # wf + dec first (tiny, unblock early PE work on the scalar ring)
    wf_row = const.tile([1, A], F32)
    nc.scalar.dma_start(out=wf_row, in_=w_wf.unsqueeze(0))
    dec_sb = const.tile([BL, D], F32)
    nc.scalar.dma_start(out=dec_sb, in_=dec)

    # wf^T: [128, at] via PE transposes of [1,128] chunks (one PSUM group)
    wfT_ps = psS.tile([128, AT], F32, tag="scores")
    for t in range(AT):
        nc.tensor.matmul(
            wfT_ps[:, t : t + 1], wf_row[0:1, t * 128 : (t + 1) * 128],
            ident[0:1, 0:1],
            is_transpose=True, start=(t == 0), stop=(t == AT - 1),
        )
    wfT = const.tile([128, AT], MMDT)
    nc.any.tensor_copy(out=wfT, in_=wfT_ps)

    # decoder hidden transposed: decT [128, dt, b]
    decT_ps = psA.tile([128, DT, BL], F32, tag="attp")
    for dt_i in range(DT):
        nc.tensor.matmul(
            decT_ps[:, dt_i, :],
            dec_sb[:, dt_i * 128 : (dt_i + 1) * 128],
            ident[0:BL, 0:BL],
            is_transpose=True, start=(dt_i == 0), stop=(dt_i == DT - 1),
        )
    decT = const.tile([128, DT, BL], MMDT)
    nc.any.tensor_copy(out=decT, in_=decT_ps)

    # Weights: direct DMA into f32r tiles (DRAM declared f32r).
    we_sb = const.tile([128, ET, A], MMDT)
    we_src = w_we.rearrange("(et p) a -> p et a", p=128)
    for c in range(4):
        sl = slice(c * (ET // 4), (c + 1) * (ET // 4))
        nc.scalar.dma_start(out=we_sb[:, sl, :], in_=we_src[:, sl, :])
    wd_sb = const.tile([128, DT, A], MMDT)
    nc.scalar.dma_start(out=wd_sb, in_=w_wd.rearrange("(dt p) a -> p dt a", p=128))
    rows_r = const.tile([1, 2, A], MMDT)
    nc.scalar.dma_start(out=rows_r[:, 0, :], in_=w_be.unsqueeze(0))
    nc.scalar.dma_start(out=rows_r[:, 1, :], in_=w_bd.unsqueeze(0))
    be_row = rows_r[:, 0, :]
    bd_row = rows_r[:, 1, :]

    # att2^T + bd + be : [128, at, b]
    att2T = const.tile([128, AT, BL], F32)
    for at_i in range(AT):
        a2_ps = psA.tile([128, BL], F32, tag="attp")
        for dt_i in range(DT):
            nc.tensor.matmul(
                a2_ps,
                wd_sb[:, dt_i, at_i * 128 : (at_i + 1) * 128],
                decT[:, dt_i, :],
                start=(dt_i == 0), stop=False,
            )
        nc.tensor.matmul(
            a2_ps, bd_row[0:1, at_i * 128 : (at_i + 1) * 128],
            ones_row[0:1, 0:BL], start=False, stop=False,
        )
        nc.tensor.matmul(
            a2_ps, be_row[0:1, at_i * 128 : (at_i + 1) * 128],
            ones_row[0:1, 0:BL], start=False, stop=True,
        )
        nc.any.tensor_copy(out=att2T[:, at_i, :], in_=a2_ps)

    nchunk = (BL * ET) // 128  # 4
    aweT = const.tile([128, nchunk, 128], F32)

    def flush_awe_chunk(t):
        # transpose columns [t*128,(t+1)*128) of awe_all and DMA them out.
        # aweT[r, t, p] = awe for flat col (b*ET+et) = t*128+r at e = et*128+p
        # DRAM addr(b, e) = (t*128+r)*128 + p
        ps = psB.tile([128, 128], F32, tag="bcast")
        nc.tensor.matmul(
            ps, awe_all[:, t * 128 : (t + 1) * 128], ident[:, :],
            is_transpose=True, start=True, stop=True,
        )
        nc.any.tensor_copy(out=aweT[:, t, :], in_=ps)
        awe_flat = bass.AP(
            tensor=awe_o.tensor, offset=awe_o.offset + t * 128 * 128,
            ap=[[128, 128], [1, 128]],
        )
        nc.scalar.dma_start(out=awe_flat, in_=aweT[:, t, :])

    # loop pools created after weight staging released its space
    xnat = ctx.enter_context(tc.tile_pool(name="xnat", bufs=2))
    xt = ctx.enter_context(tc.tile_pool(name="xt", bufs=18))
    tanhp = ctx.enter_context(tc.tile_pool(name="tanh", bufs=5))
    small = ctx.enter_context(tc.tile_pool(name="small", bufs=2))
    scr = ctx.enter_context(tc.tile_pool(name="scr", bufs=2))

    # awe accumulator: col (b*ET + et), partition p -> e = et*128 + p
    awe_all = const.tile([128, BL * ET], F32)

    # ---------------- main loop over batch pairs (lookahead-2 prefetch) ----
    def load_pair(p2):
        b0, b1 = 2 * p2, 2 * p2 + 1
        xa0 = xnat.tile([128, E], MMDT, tag="xa0")
        xb0 = xnat.tile([L1, E], MMDT, tag="xb0")
        xa1 = xnat.tile([128, E], MMDT, tag="xa1")
        xb1 = xnat.tile([L1, E], MMDT, tag="xb1")
        nc.sync.dma_start(out=xa0, in_=enc[b0, 0:L0, :])
        nc.sync.dma_start(out=xb0, in_=enc[b0, L0:L, :])
        nc.sync.dma_start(out=xa1, in_=enc[b1, 0:L0, :])
        nc.sync.dma_start(out=xb1, in_=enc[b1, L0:L, :])
        mski = small.tile([1, 2, L], I32, tag="mski")
        nc.sync.dma_start(out=mski, in_=msk[b0 : b0 + 2, :].unsqueeze(0))
        return xa0, xb0, xa1, xb1, mski

    prefetched = {0: load_pair(0), 1: load_pair(1)}

    for p2 in range(NPAIR):
        b0, b1 = 2 * p2, 2 * p2 + 1
        if p2 + 2 < NPAIR:
            prefetched[p2 + 2] = load_pair(p2 + 2)
        xa0, xb0, xa1, xb1, mski = prefetched.pop(p2)

        # additive mask for the pair at partition 0: (m-1)*1e9
        mska = small.tile([1, W2], F32, tag="mska")
        nc.scalar.activation(
            out=mska, in_=mski.rearrange("p b l -> p (b l)"),
            func=mybir.ActivationFunctionType.Copy, bias=-1e9, scale=1e9,
        )

        # X^T pair tiles [128, 392]
        xts = []
        for et in range(ET):
            tp = psT.tile([128, W2], MMDT, tag="psT")
            srcs = (
                (xa0[:, et * 128 : (et + 1) * 128], 0, L0),
                (xb0[:, et * 128 : (et + 1) * 128], L0, L1),
                (xa1[:, et * 128 : (et + 1) * 128], L, L0),
                (xb1[:, et * 128 : (et + 1) * 128], L + L0, L1),
            )
            for i, (src, off, ln) in enumerate(srcs):
                nc.tensor.matmul(
                    tp[:, off : off + ln], src, ident_r[0:ln, 0:ln],
                    is_transpose=True, start=(i == 0), stop=(i == 3),
                )
            xt_t = xt.tile([128, W2], MMDT, tag="xt")
            nc.any.tensor_copy(out=xt_t, in_=tp)
            xts.append(xt_t)

        # att1^T pair-packed + tanh(+att2 bias)
        tanhs = []
        for at_i in range(AT):
            ap_ps = psA.tile([128, W2], F32, tag="attp")
            for et in range(ET):
                nc.tensor.matmul(
                    ap_ps,
                    we_sb[:, et, at_i * 128 : (at_i + 1) * 128],
                    xts[et][:],
                    start=(et == 0), stop=(et == ET - 1),
                )
            th = tanhp.tile([128, W2], MMDT, tag="tanh")
            for h, b in ((0, b0), (1, b1)):
                nc.scalar.activation(
                    out=th[:, h * L : (h + 1) * L], in_=ap_ps[:, h * L : (h + 1) * L],
                    func=mybir.ActivationFunctionType.Tanh,
                    bias=att2T[:, at_i, b : b + 1], scale=1.0,
                )
            tanhs.append(th)

        # scores [1, 392] at partition 0
        sc_ps = psS.tile([1, W2], F32, tag="scores")
        for at_i in range(AT):
            nc.tensor.matmul(
                sc_ps, wfT[:, at_i : at_i + 1], tanhs[at_i][:],
                start=(at_i == 0), stop=(at_i == AT - 1),
            )

        # masked softmax per 196-segment
        s_sb = small.tile([1, W2], F32, tag="s_sb")
        nc.vector.tensor_add(s_sb, sc_ps, mska)
        nmax = small.tile([1, 2], F32, tag="nmax")
        nc.vector.tensor_reduce(
            out=nmax, in_=s_sb.rearrange("p (b l) -> p b l", b=2),
            axis=mybir.AxisListType.X, op=mybir.AluOpType.max, negate=True,
        )
        e_sb = small.tile([1, W2], F32, tag="e_sb")
        sums = small.tile([1, 2], F32, tag="sums")
        for h in range(2):
            nc.scalar.activation(
                out=e_sb[0:1, h * L : (h + 1) * L],
                in_=s_sb[0:1, h * L : (h + 1) * L],
                func=mybir.ActivationFunctionType.Exp,
                bias=nmax[0:1, h : h + 1], scale=1.0,
                accum_out=sums[0:1, h : h + 1],
            )
        rec = small.tile([1, 2], F32, tag="rec")
        nc.vector.reciprocal(rec, sums)
        alp_t = small.tile([1, W2], F32, tag="alp")
        for h in range(2):
            nc.vector.tensor_scalar_mul(
                alp_t[0:1, h * L : (h + 1) * L],
                e_sb[0:1, h * L : (h + 1) * L],
                rec[0:1, h : h + 1],
            )
        nc.scalar.dma_start(
            out=alp_o[b0 : b0 + 2, :].unsqueeze(0),
            in_=alp_t.rearrange("p (b l) -> p b l", b=2),
        )

        # broadcast alpha across partitions (K=1 matmul, via rounded copy)
        alp_r = small.tile([1, W2], MMDT, tag="alp_r")
        nc.vector.tensor_copy(out=alp_r, in_=alp_t)
        bc_ps = psB.tile([128, W2], F32, tag="bcast")
        nc.tensor.matmul(
            bc_ps, ones_row[0:1, 0:128], alp_r[0:1, :],
            start=True, stop=True,
        )

        # awe: fused multiply + accumulate over l on DVE
        for et in range(ET):
            for h, b in ((0, b0), (1, b1)):
                junk = scr.tile([128, L], F32, tag="junk")
                nc.vector.scalar_tensor_tensor(
                    out=junk,
                    in0=xts[et][:, h * L : (h + 1) * L].bitcast(F32),
                    scalar=1.0,
                    in1=bc_ps[:, h * L : (h + 1) * L],
                    op0=mybir.AluOpType.mult,
                    op1=mybir.AluOpType.mult,
                    accum_out=awe_all[:, b * ET + et : b * ET + et + 1],
                )

        if (p2 + 1) % 4 == 0:
            flush_awe_chunk((p2 + 1) // 4 - 1)


_CACHE = {}


def _get_nc():
    if "nc" in _CACHE:
        return _CACHE["nc"]
    nc = bacc.Bacc("TRN2", target_bir_lowering=False, debug=False,
                  num_devices=1)
    aps = {
        "enc": nc.dram_tensor("enc", [BL, L, E], MMDT, kind="ExternalInput").ap(),
        "dec": nc.dram_tensor("dec", [BL, D], F32, kind="ExternalInput").ap(),
        "msk": nc.dram_tensor("msk", [BL, L], I32, kind="ExternalInput").ap(),
        "We": nc.dram_tensor("We", [E, A], MMDT, kind="ExternalInput").ap(),
        "be": nc.dram_tensor("be", [A], MMDT, kind="ExternalInput").ap(),
        "Wd": nc.dram_tensor("Wd", [D, A], MMDT, kind="ExternalInput").ap(),
        "bd": nc.dram_tensor("bd", [A], MMDT, kind="ExternalInput").ap(),
        "wf": nc.dram_tensor("wf", [A], F32, kind="ExternalInput").ap(),
        "awe": nc.dram_tensor("awe", [BL, E], F32, kind="ExternalOutput").ap(),
        "alp": nc.dram_tensor("alp", [BL, L], F32, kind="ExternalOutput").ap(),
    }
    with tile.TileContext(nc) as tc:
        with ExitStack() as ctx:
            build_kernel(ctx, tc, aps)
    nc.compile()
    _CACHE["nc"] = nc
    return nc


def make_in_maps(encoder_out, decoder_hidden, mask1, We, be, Wd, bd, wf):
    enc = np.ascontiguousarray(np.asarray(encoder_out, dtype=np.float32))
    dec = np.ascontiguousarray(np.asarray(decoder_hidden, dtype=np.float32))
    msk = np.ascontiguousarray(np.asarray(mask1, dtype=np.int32))
    wts = {
        "We": np.ascontiguousarray(np.asarray(We, dtype=np.float32)),
        "be": np.ascontiguousarray(np.asarray(be, dtype=np.float32)),
        "Wd": np.ascontiguousarray(np.asarray(Wd, dtype=np.float32)),
        "bd": np.ascontiguousarray(np.asarray(bd, dtype=np.float32)),
        "wf": np.ascontiguousarray(np.asarray(wf, dtype=np.float32)),
    }
    in_maps = []
    for i in range(NCORES):
        sl = slice(i * BL, (i + 1) * BL)
        in_maps.append({
            "enc": enc[sl], "dec": dec[sl], "msk": msk[sl], **wts,
        })
    return in_maps


def kernel(encoder_out, decoder_hidden, mask1, We, be, Wd, bd, wf, bf=None,
           **_ignored):
    from concourse.bass_utils import run_bass_kernel_spmd

    nc = _get_nc()
    in_maps = make_in_maps(encoder_out, decoder_hidden, mask1, We, be, Wd, bd, wf)
    res = run_bass_kernel_spmd(nc, in_maps, core_ids=list(range(NCORES)))
    awe = np.concatenate([r["awe"] for r in res.results], axis=0)
    alp = np.concatenate([r["alp"] for r in res.results], axis=0)
    return awe, alp


if __name__ == "__main__":
    # smoke build
    _get_nc()
    print("build OK")


# revision 20
# speedup vs baseline: 177.1565x; 177.1565x over previous
"""Bahdanau-attention kernel for Trainium2 (8 NeuronCores, batch-sharded).

Reference computation (per batch b):
  att1 = enc[b] @ We + be                    # [L, A]
  att2 = dec[b] @ Wd + bd                    # [A]
  att  = tanh(att1 + att2) @ wf (+ bf)       # [L]   (bf cancels in softmax)
  att  = where(mask==0, -1e9, att)
  alpha = softmax(att)                       # [L]
  awe  = alpha @ enc[b]                      # [E]
Outputs: (awe [B,E] f32, alpha [B,L] f32)

Per-core strategy (32 local batches, processed in pairs):
  - DMA enc natural [l, e] tiles; PE-transpose to X^T [e, l] pair tiles.
  - att1^T [a, l] via PE: stationary = We tile (natural layout), moving = X^T
    pair tile (N=392 >= 256 so float32r streams at 1 cycle/row, full fp32 bits).
  - tanh on ACT reading PSUM, bias = column of att2^T (bd+be folded in).
  - scores via PE (wf^T stationary, M=1) -> [1, 392] at partition 0;
    additive-masked softmax there.
  - awe via DVE scalar_tensor_tensor with fused accum (sum over l), weighting
    X^T tiles with a PE-broadcast alpha.
"""

import sys

for _p in ("/opt/trn_rl_repo",):
    if _p not in sys.path:
        sys.path.insert(0, _p)

from contextlib import ExitStack

import numpy as np

import concourse.bass as bass
import concourse.tile as tile
from concourse import bacc
from concourse import mybir
from concourse.masks import make_identity

B, L, E, D, A = 256, 196, 2048, 1024, 512
NCORES = 8
BL = B // NCORES          # 32 batches per core
NPAIR = BL // 2           # 16 pairs
ET, AT, DT = E // 128, A // 128, D // 128   # 16, 4, 8
L0, L1 = 128, L - 128     # 128, 68
W2 = 2 * L                # 392 (pair-packed free width)

F32 = mybir.dt.float32
F32R = mybir.dt.float32r
I32 = mybir.dt.int32

# Streaming dtype for the big matmuls: "f32r" (full fp32 bits, fast streaming
# mode) or "f32" (4x slower). Flip if f32r misbehaves on HW.
MM_MODE = "f32r"
MMDT = F32R if MM_MODE == "f32r" else F32


def build_kernel(ctx: ExitStack, tc: tile.TileContext, aps):
    nc = tc.nc
    enc, dec, msk = aps["enc"], aps["dec"], aps["msk"]
    w_we, w_be, w_wd, w_bd, w_wf = aps["We"], aps["be"], aps["Wd"], aps["bd"], aps["wf"]
    awe_o, alp_o = aps["awe"], aps["alp"]

    const = ctx.enter_context(tc.tile_pool(name="const", bufs=1))
    # PSUM: 8 banks total = 2 + 3 + 1 + 2
    psT = ctx.enter_context(tc.tile_pool(name="psT", bufs=2, space="PSUM"))
    psA = ctx.enter_context(tc.tile_pool(name="psA", bufs=3, space="PSUM"))
    psS = ctx.enter_context(tc.tile_pool(name="psS", bufs=1, space="PSUM"))
    psB = ctx.enter_context(tc.tile_pool(name="psB", bufs=2, space="PSUM"))

    # ---------------- constants / preamble ----------------
    ident = const.tile([128, 128], F32)
    make_identity(nc, ident)
    ident_r = const.tile([128, 128], MMDT)
    nc.vector.tensor_copy(out=ident_r, in_=ident)

    ones_st = const.tile([1, 128], F32)
    nc.vector.memset(ones_st, 1.0)
    ones_row = const.tile([1, 128], MMDT)
    nc.vector.tensor_copy(out=ones_row, in_=ones_st)

    # wf + dec first (tiny, unblock early PE work on the scalar ring)
    wf_row = const.tile([1, A], F32)
    nc.scalar.dma_start(out=wf_row, in_=w_wf.unsqueeze(0))
    dec_sb = const.tile([BL, D], F32)
    nc.scalar.dma_start(out=dec_sb, in_=dec)

    # wf^T: [128, at] via PE transposes of [1,128] chunks (one PSUM group)
    wfT_ps = psS.tile([128, AT], F32, tag="scores")
    for t in range(AT):
        nc.tensor.matmul(
            wfT_ps[:, t : t + 1], wf_row[0:1, t * 128 : (t + 1) * 128],
            ident[0:1, 0:1],
            is_transpose=True, start=(t == 0), stop=(t == AT - 1),
        )
    wfT = const.tile([128, AT], MMDT)
    nc.any.tensor_copy(out=wfT, in_=wfT_ps)

    # decoder hidden transposed: decT [128, dt, b]
    decT_ps = psA.tile([128, DT, BL], F32, tag="attp")
    for dt_i in range(DT):
        nc.tensor.matmul(
            decT_ps[:, dt_i, :],
            dec_sb[:, dt_i * 128 : (dt_i + 1) * 128],
            ident[0:BL, 0:BL],
            is_transpose=True, start=(dt_i == 0), stop=(dt_i == DT - 1),
        )
    decT = const.tile([128, DT, BL], MMDT)
    nc.any.tensor_copy(out=decT, in_=decT_ps)

    # Weights: direct DMA into f32r tiles (DRAM declared f32r).
    we_sb = const.tile([128, ET, A], MMDT)
    we_src = w_we.rearrange("(et p) a -> p et a", p=128)
    for c in range(4):
        sl = slice(c * (ET // 4), (c + 1) * (ET // 4))
        nc.scalar.dma_start(out=we_sb[:, sl, :], in_=we_src[:, sl, :])
    wd_sb = const.tile([128, DT, A], MMDT)
    nc.scalar.dma_start(out=wd_sb, in_=w_wd.rearrange("(dt p) a -> p dt a", p=128))
    rows_r = const.tile([1, 2, A], MMDT)
    nc.scalar.dma_start(out=rows_r[:, 0, :], in_=w_be.unsqueeze(0))
    nc.scalar.dma_start(out=rows_r[:, 1, :], in_=w_bd.unsqueeze(0))
    be_row = rows_r[:, 0, :]
    bd_row = rows_r[:, 1, :]

    # att2^T + bd + be : [128, at, b]
    att2T = const.tile([128, AT, BL], F32)
    for at_i in range(AT):
        a2_ps = psA.tile([128, BL], F32, tag="attp")
        for dt_i in range(DT):
            nc.tensor.matmul(
                a2_ps,
                wd_sb[:, dt_i, at_i * 128 : (at_i + 1) * 128],
                decT[:, dt_i, :],
                start=(dt_i == 0), stop=False,
            )
        nc.tensor.matmul(
            a2_ps, bd_row[0:1, at_i * 128 : (at_i + 1) * 128],
            ones_row[0:1, 0:BL], start=False, stop=False,
        )
        nc.tensor.matmul(
            a2_ps, be_row[0:1, at_i * 128 : (at_i + 1) * 128],
            ones_row[0:1, 0:BL], start=False, stop=True,
        )
        nc.any.tensor_copy(out=att2T[:, at_i, :], in_=a2_ps)

    nchunk = (BL * ET) // 128  # 4
    aweT = const.tile([128, nchunk, 128], F32)

    def flush_awe_chunk(t):
        # transpose columns [t*128,(t+1)*128) of awe_all and DMA them out.
        # aweT[r, t, p] = awe for flat col (b*ET+et) = t*128+r at e = et*128+p
        # DRAM addr(b, e) = (t*128+r)*128 + p
        ps = psB.tile([128, 128], F32, tag="bcast")
        nc.tensor.matmul(
            ps, awe_all[:, t * 128 : (t + 1) * 128], ident[:, :],
            is_transpose=True, start=True, stop=True,
        )
        nc.any.tensor_copy(out=aweT[:, t, :], in_=ps)
        awe_flat = bass.AP(
            tensor=awe_o.tensor, offset=awe_o.offset + t * 128 * 128,
            ap=[[128, 128], [1, 128]],
        )
        nc.scalar.dma_start(out=awe_flat, in_=aweT[:, t, :])

    # loop pools created after weight staging released its space
    xnat = ctx.enter_context(tc.tile_pool(name="xnat", bufs=2))
    xt = ctx.enter_context(tc.tile_pool(name="xt", bufs=18))
    tanhp = ctx.enter_context(tc.tile_pool(name="tanh", bufs=5))
    small = ctx.enter_context(tc.tile_pool(name="small", bufs=2))
    scr = ctx.enter_context(tc.tile_pool(name="scr", bufs=2))

    # awe accumulator: col (b*ET + et), partition p -> e = et*128 + p
    awe_all = const.tile([128, BL * ET], F32)

    # ---------------- main loop over batch pairs (lookahead-2 prefetch) ----
    def load_pair(p2):
        b0, b1 = 2 * p2, 2 * p2 + 1
        xa0 = xnat.tile([128, E], MMDT, tag="xa0")
        xb0 = xnat.tile([L1, E], MMDT, tag="xb0")
        xa1 = xnat.tile([128, E], MMDT, tag="xa1")
        xb1 = xnat.tile([L1, E], MMDT, tag="xb1")
        nc.sync.dma_start(out=xa0, in_=enc[b0, 0:L0, :])
        nc.sync.dma_start(out=xb0, in_=enc[b0, L0:L, :])
        nc.sync.dma_start(out=xa1, in_=enc[b1, 0:L0, :])
        nc.sync.dma_start(out=xb1, in_=enc[b1, L0:L, :])
        mski = small.tile([1, 2, L], I32, tag="mski")
        nc.sync.dma_start(out=mski, in_=msk[b0 : b0 + 2, :].unsqueeze(0))
        return xa0, xb0, xa1, xb1, mski

    # REPEAT>1 re-runs the whole pair loop (timing harness only; outputs are
    # simply overwritten each repetition).
    import os as _os
    repeat = int(_os.environ.get("BASS_KERNEL_REPEAT", "1"))
    niter = NPAIR * repeat

    prefetched = {0: load_pair(0), 1: load_pair(1)}

    for it in range(niter):
        p2 = it % NPAIR
        b0, b1 = 2 * p2, 2 * p2 + 1
        if it + 2 < niter:
            prefetched[it + 2] = load_pair((it + 2) % NPAIR)
        xa0, xb0, xa1, xb1, mski = prefetched.pop(it)

        # additive mask for the pair at partition 0: (m-1)*1e9
        mska = small.tile([1, W2], F32, tag="mska")
        nc.scalar.activation(
            out=mska, in_=mski.rearrange("p b l -> p (b l)"),
            func=mybir.ActivationFunctionType.Copy, bias=-1e9, scale=1e9,
        )

        # X^T pair tiles [128, 392]
        xts = []
        for et in range(ET):
            tp = psT.tile([128, W2], MMDT, tag="psT")
            srcs = (
                (xa0[:, et * 128 : (et + 1) * 128], 0, L0),
                (xb0[:, et * 128 : (et + 1) * 128], L0, L1),
                (xa1[:, et * 128 : (et + 1) * 128], L, L0),
                (xb1[:, et * 128 : (et + 1) * 128], L + L0, L1),
            )
            for i, (src, off, ln) in enumerate(srcs):
                nc.tensor.matmul(
                    tp[:, off : off + ln], src, ident_r[0:ln, 0:ln],
                    is_transpose=True, start=(i == 0), stop=(i == 3),
                )
            xt_t = xt.tile([128, W2], MMDT, tag="xt")
            nc.any.tensor_copy(out=xt_t, in_=tp)
            xts.append(xt_t)

        # att1^T pair-packed + tanh(+att2 bias)
        tanhs = []
        for at_i in range(AT):
            ap_ps = psA.tile([128, W2], F32, tag="attp")
            for et in range(ET):
                nc.tensor.matmul(
                    ap_ps,
                    we_sb[:, et, at_i * 128 : (at_i + 1) * 128],
                    xts[et][:],
                    start=(et == 0), stop=(et == ET - 1),
                )
            th = tanhp.tile([128, W2], MMDT, tag="tanh")
            for h, b in ((0, b0), (1, b1)):
                nc.scalar.activation(
                    out=th[:, h * L : (h + 1) * L], in_=ap_ps[:, h * L : (h + 1) * L],
                    func=mybir.ActivationFunctionType.Tanh,
                    bias=att2T[:, at_i, b : b + 1], scale=1.0,
                )
            tanhs.append(th)

        # scores [1, 392] at partition 0
        sc_ps = psS.tile([1, W2], F32, tag="scores")
        for at_i in range(AT):
            nc.tensor.matmul(
                sc_ps, wfT[:, at_i : at_i + 1], tanhs[at_i][:],
                start=(at_i == 0), stop=(at_i == AT - 1),
            )

        # masked softmax per 196-segment
        s_sb = small.tile([1, W2], F32, tag="s_sb")
        nc.vector.tensor_add(s_sb, sc_ps, mska)
        nmax = small.tile([1, 2], F32, tag="nmax")
        nc.vector.tensor_reduce(
            out=nmax, in_=s_sb.rearrange("p (b l) -> p b l", b=2),
            axis=mybir.AxisListType.X, op=mybir.AluOpType.max, negate=True,
        )
        e_sb = small.tile([1, W2], F32, tag="e_sb")
        sums = small.tile([1, 2], F32, tag="sums")
        for h in range(2):
            nc.scalar.activation(
                out=e_sb[0:1, h * L : (h + 1) * L],
                in_=s_sb[0:1, h * L : (h + 1) * L],
                func=mybir.ActivationFunctionType.Exp,
                bias=nmax[0:1, h : h + 1], scale=1.0,
                accum_out=sums[0:1, h : h + 1],
            )
        rec = small.tile([1, 2], F32, tag="rec")
        nc.vector.reciprocal(rec, sums)
        alp_t = small.tile([1, W2], F32, tag="alp")
        for h in range(2):
            nc.vector.tensor_scalar_mul(
                alp_t[0:1, h * L : (h + 1) * L],
                e_sb[0:1, h * L : (h + 1) * L],
                rec[0:1, h : h + 1],
            )
        nc.scalar.dma_start(
            out=alp_o[b0 : b0 + 2, :].unsqueeze(0),
            in_=alp_t.rearrange("p (b l) -> p b l", b=2),
        )

        # broadcast alpha across partitions (K=1 matmul, via rounded copy)
        alp_r = small.tile([1, W2], MMDT, tag="alp_r")
        nc.vector.tensor_copy(out=alp_r, in_=alp_t)
        bc_ps = psB.tile([128, W2], F32, tag="bcast")
        nc.tensor.matmul(
            bc_ps, ones_row[0:1, 0:128], alp_r[0:1, :],
            start=True, stop=True,
        )

        # awe: fused multiply + accumulate over l on DVE
        for et in range(ET):
            for h, b in ((0, b0), (1, b1)):
                junk = scr.tile([128, L], F32, tag="junk")
                nc.vector.scalar_tensor_tensor(
                    out=junk,
                    in0=xts[et][:, h * L : (h + 1) * L].bitcast(F32),
                    scalar=1.0,
                    in1=bc_ps[:, h * L : (h + 1) * L],
                    op0=mybir.AluOpType.mult,
                    op1=mybir.AluOpType.mult,
                    accum_out=awe_all[:, b * ET + et : b * ET + et + 1],
                )

        if (p2 + 1) % 4 == 0:
            flush_awe_chunk((p2 + 1) // 4 - 1)



_CACHE = {}


def _get_nc():
    if "nc" in _CACHE:
        return _CACHE["nc"]
    nc = bacc.Bacc("TRN2", target_bir_lowering=False, debug=False,
                  num_devices=1)
    aps = {
        "enc": nc.dram_tensor("enc", [BL, L, E], MMDT, kind="ExternalInput").ap(),
        "dec": nc.dram_tensor("dec", [BL, D], F32, kind="ExternalInput").ap(),
        "msk": nc.dram_tensor("msk", [BL, L], I32, kind="ExternalInput").ap(),
        "We": nc.dram_tensor("We", [E, A], MMDT, kind="ExternalInput").ap(),
        "be": nc.dram_tensor("be", [A], MMDT, kind="ExternalInput").ap(),
        "Wd": nc.dram_tensor("Wd", [D, A], MMDT, kind="ExternalInput").ap(),
        "bd": nc.dram_tensor("bd", [A], MMDT, kind="ExternalInput").ap(),
        "wf": nc.dram_tensor("wf", [A], F32, kind="ExternalInput").ap(),
        "awe": nc.dram_tensor("awe", [BL, E], F32, kind="ExternalOutput").ap(),
        "alp": nc.dram_tensor("alp", [BL, L], F32, kind="ExternalOutput").ap(),
    }
    with tile.TileContext(nc) as tc:
        with ExitStack() as ctx:
            build_kernel(ctx, tc, aps)
    nc.compile()
    _CACHE["nc"] = nc
    return nc


def make_in_maps(encoder_out, decoder_hidden, mask1, We, be, Wd, bd, wf):
    enc = np.ascontiguousarray(np.asarray(encoder_out, dtype=np.float32))
    dec = np.ascontiguousarray(np.asarray(decoder_hidden, dtype=np.float32))
    msk = np.ascontiguousarray(np.asarray(mask1, dtype=np.int32))
    wts = {
        "We": np.ascontiguousarray(np.asarray(We, dtype=np.float32)),
        "be": np.ascontiguousarray(np.asarray(be, dtype=np.float32)),
        "Wd": np.ascontiguousarray(np.asarray(Wd, dtype=np.float32)),
        "bd": np.ascontiguousarray(np.asarray(bd, dtype=np.float32)),
        "wf": np.ascontiguousarray(np.asarray(wf, dtype=np.float32)),
    }
    in_maps = []
    for i in range(NCORES):
        sl = slice(i * BL, (i + 1) * BL)
        in_maps.append({
            "enc": enc[sl], "dec": dec[sl], "msk": msk[sl], **wts,
        })
    return in_maps


def kernel(encoder_out, decoder_hidden, mask1, We, be, Wd, bd, wf, bf=None,
           **_ignored):
    from concourse.bass_utils import run_bass_kernel_spmd

    nc = _get_nc()
    in_maps = make_in_maps(encoder_out, decoder_hidden, mask1, We, be, Wd, bd, wf)
    res = run_bass_kernel_spmd(nc, in_maps, core_ids=list(range(NCORES)))
    awe = np.concatenate([r["awe"] for r in res.results], axis=0)
    alp = np.concatenate([r["alp"] for r in res.results], axis=0)
    return awe, alp


if __name__ == "__main__":
    # smoke build
    _get_nc()
    print("build OK")


# revision 22
# speedup vs baseline: 501.7471x; 2.8322x over previous
"""Bahdanau-attention kernel for Trainium2 (8 NeuronCores, batch-sharded).

Reference computation (per batch b):
  att1 = enc[b] @ We + be                    # [L, A]
  att2 = dec[b] @ Wd + bd                    # [A]
  att  = tanh(att1 + att2) @ wf (+ bf)       # [L]   (bf cancels in softmax)
  att  = where(mask==0, -1e9, att)
  alpha = softmax(att)                       # [L]
  awe  = alpha @ enc[b]                      # [E]
Outputs: (awe [B,E] f32, alpha [B,L] f32)

Per-core strategy (32 local batches, processed in pairs):
  - DMA enc natural [l, e] tiles; PE-transpose to X^T [e, l] pair tiles.
  - att1^T [a, l] via PE: stationary = We tile (natural layout), moving = X^T
    pair tile (N=392 >= 256 so float32r streams at 1 cycle/row, full fp32 bits).
  - tanh on ACT reading PSUM, bias = column of att2^T (bd+be folded in).
  - scores via PE (wf^T stationary, M=1) -> [1, 392] at partition 0;
    additive-masked softmax there.
  - awe via DVE scalar_tensor_tensor with fused accum (sum over l), weighting
    X^T tiles with a PE-broadcast alpha.
"""

import sys

for _p in ("/opt/trn_rl_repo",):
    if _p not in sys.path:
        sys.path.insert(0, _p)

from contextlib import ExitStack

import numpy as np

import concourse.bass as bass
import concourse.tile as tile
from concourse import bacc
from concourse import mybir
from concourse.masks import make_identity

B, L, E, D, A = 256, 196, 2048, 1024, 512
NCORES = 8
BL = B // NCORES          # 32 batches per core
NPAIR = BL // 2           # 16 pairs
ET, AT, DT = E // 128, A // 128, D // 128   # 16, 4, 8
L0, L1 = 128, L - 128     # 128, 68
W2 = 2 * L                # 392 (pair-packed free width)

F32 = mybir.dt.float32
F32R = mybir.dt.float32r
I32 = mybir.dt.int32

# Streaming dtype for the big matmuls: "f32r" (full fp32 bits, fast streaming
# mode) or "f32" (4x slower). Flip if f32r misbehaves on HW.
MM_MODE = "f32r"
MMDT = F32R if MM_MODE == "f32r" else F32


def build_kernel(ctx: ExitStack, tc: tile.TileContext, aps):
    nc = tc.nc
    enc, dec, msk = aps["enc"], aps["dec"], aps["msk"]
    w_we, w_be, w_wd, w_bd, w_wf = aps["We"], aps["be"], aps["Wd"], aps["bd"], aps["wf"]
    awe_o, alp_o = aps["awe"], aps["alp"]

    const = ctx.enter_context(tc.tile_pool(name="const", bufs=1))
    # PSUM: 8 banks total = 2 + 3 + 1 + 2
    psT = ctx.enter_context(tc.tile_pool(name="psT", bufs=2, space="PSUM"))
    psA = ctx.enter_context(tc.tile_pool(name="psA", bufs=4, space="PSUM"))
    psS = ctx.enter_context(tc.tile_pool(name="psS", bufs=1, space="PSUM"))
    psB = ctx.enter_context(tc.tile_pool(name="psB", bufs=1, space="PSUM"))

    # ---------------- constants / preamble ----------------
    ident = const.tile([128, 128], F32)
    make_identity(nc, ident)
    ident_r = const.tile([128, 128], MMDT)
    nc.vector.tensor_copy(out=ident_r, in_=ident)

    ones_st = const.tile([1, 128], F32)
    nc.vector.memset(ones_st, 1.0)
    ones_row = const.tile([1, 128], MMDT)
    nc.vector.tensor_copy(out=ones_row, in_=ones_st)

    # wf + dec first (tiny, unblock early PE work on the scalar ring)
    wf_row = const.tile([1, A], F32)
    nc.scalar.dma_start(out=wf_row, in_=w_wf.unsqueeze(0))
    dec_sb = const.tile([BL, D], F32)
    nc.scalar.dma_start(out=dec_sb, in_=dec)

    # wf^T: [128, at] via PE transposes of [1,128] chunks (one PSUM group)
    wfT_ps = psS.tile([128, AT], F32, tag="scores")
    for t in range(AT):
        nc.tensor.matmul(
            wfT_ps[:, t : t + 1], wf_row[0:1, t * 128 : (t + 1) * 128],
            ident[0:1, 0:1],
            is_transpose=True, start=(t == 0), stop=(t == AT - 1),
        )
    wfT = const.tile([128, AT], MMDT)
    nc.any.tensor_copy(out=wfT, in_=wfT_ps)

    # decoder hidden transposed: decT [128, dt, b]
    decT_ps = psA.tile([128, DT, BL], F32, tag="attp")
    for dt_i in range(DT):
        nc.tensor.matmul(
            decT_ps[:, dt_i, :],
            dec_sb[:, dt_i * 128 : (dt_i + 1) * 128],
            ident[0:BL, 0:BL],
            is_transpose=True, start=(dt_i == 0), stop=(dt_i == DT - 1),
        )
    decT = const.tile([128, DT, BL], MMDT)
    nc.any.tensor_copy(out=decT, in_=decT_ps)

    # Weights: direct DMA into f32r tiles (DRAM declared f32r).
    we_sb = const.tile([128, ET, A], MMDT)
    we_src = w_we.rearrange("(et p) a -> p et a", p=128)
    for c in range(4):
        sl = slice(c * (ET // 4), (c + 1) * (ET // 4))
        nc.scalar.dma_start(out=we_sb[:, sl, :], in_=we_src[:, sl, :])
    wd_sb = const.tile([128, DT, A], MMDT)
    nc.scalar.dma_start(out=wd_sb, in_=w_wd.rearrange("(dt p) a -> p dt a", p=128))
    rows_r = const.tile([1, 2, A], MMDT)
    nc.scalar.dma_start(out=rows_r[:, 0, :], in_=w_be.unsqueeze(0))
    nc.scalar.dma_start(out=rows_r[:, 1, :], in_=w_bd.unsqueeze(0))
    be_row = rows_r[:, 0, :]
    bd_row = rows_r[:, 1, :]

    # att2^T + bd + be : [128, at, b]
    att2T = const.tile([128, AT, BL], F32)
    for at_i in range(AT):
        a2_ps = psA.tile([128, BL], F32, tag="attp")
        for dt_i in range(DT):
            nc.tensor.matmul(
                a2_ps,
                wd_sb[:, dt_i, at_i * 128 : (at_i + 1) * 128],
                decT[:, dt_i, :],
                start=(dt_i == 0), stop=False,
            )
        nc.tensor.matmul(
            a2_ps, bd_row[0:1, at_i * 128 : (at_i + 1) * 128],
            ones_row[0:1, 0:BL], start=False, stop=False,
        )
        nc.tensor.matmul(
            a2_ps, be_row[0:1, at_i * 128 : (at_i + 1) * 128],
            ones_row[0:1, 0:BL], start=False, stop=True,
        )
        nc.any.tensor_copy(out=att2T[:, at_i, :], in_=a2_ps)

    nchunk = (BL * ET) // 128  # 4
    aweT = const.tile([128, nchunk, 128], F32)

    def flush_awe_chunk(t):
        # transpose columns [t*128,(t+1)*128) of awe_all and DMA them out.
        # aweT[r, t, p] = awe for flat col (b*ET+et) = t*128+r at e = et*128+p
        # DRAM addr(b, e) = (t*128+r)*128 + p
        ps = psB.tile([128, 128], F32, tag="bcast")
        nc.tensor.matmul(
            ps, awe_all[:, t * 128 : (t + 1) * 128], ident[:, :],
            is_transpose=True, start=True, stop=True,
        )
        nc.any.tensor_copy(out=aweT[:, t, :], in_=ps)
        awe_flat = bass.AP(
            tensor=awe_o.tensor, offset=awe_o.offset + t * 128 * 128,
            ap=[[128, 128], [1, 128]],
        )
        nc.scalar.dma_start(out=awe_flat, in_=aweT[:, t, :])

    # loop pools created after weight staging released its space
    xnat = ctx.enter_context(tc.tile_pool(name="xnat", bufs=2))
    xt = ctx.enter_context(tc.tile_pool(name="xt", bufs=20))
    tanhp = ctx.enter_context(tc.tile_pool(name="tanh", bufs=4))
    small = ctx.enter_context(tc.tile_pool(name="small", bufs=2))
    scr = ctx.enter_context(tc.tile_pool(name="scr", bufs=2))

    # awe accumulator: col (b*ET + et), partition p -> e = et*128 + p
    awe_all = const.tile([128, BL * ET], F32)

    # ---------------- main loop over batch pairs (lookahead-2 prefetch) ----
    def load_pair(p2):
        b0, b1 = 2 * p2, 2 * p2 + 1
        xa0 = xnat.tile([128, E], MMDT, tag="xa0")
        xb0 = xnat.tile([L1, E], MMDT, tag="xb0")
        xa1 = xnat.tile([128, E], MMDT, tag="xa1")
        xb1 = xnat.tile([L1, E], MMDT, tag="xb1")
        nc.sync.dma_start(out=xa0, in_=enc[b0, 0:L0, :])
        nc.sync.dma_start(out=xb0, in_=enc[b0, L0:L, :])
        nc.sync.dma_start(out=xa1, in_=enc[b1, 0:L0, :])
        nc.sync.dma_start(out=xb1, in_=enc[b1, L0:L, :])
        mski = small.tile([1, 2, L], I32, tag="mski")
        nc.sync.dma_start(out=mski, in_=msk[b0 : b0 + 2, :].unsqueeze(0))
        return xa0, xb0, xa1, xb1, mski

    # REPEAT>1 re-runs the whole pair loop (timing harness only; outputs are
    # simply overwritten each repetition).
    import os as _os
    repeat = int(_os.environ.get("BASS_KERNEL_REPEAT", "1"))
    niter = NPAIR * repeat

    prefetched = {0: load_pair(0), 1: load_pair(1)}

    for it in range(niter):
        p2 = it % NPAIR
        b0, b1 = 2 * p2, 2 * p2 + 1
        if it + 2 < niter:
            prefetched[it + 2] = load_pair((it + 2) % NPAIR)
        xa0, xb0, xa1, xb1, mski = prefetched.pop(it)

        # additive mask for the pair at partition 0: (m-1)*1e9
        mska = small.tile([1, W2], F32, tag="mska")
        nc.scalar.activation(
            out=mska, in_=mski.rearrange("p b l -> p (b l)"),
            func=mybir.ActivationFunctionType.Copy, bias=-1e9, scale=1e9,
        )

        # X^T pair tiles [128, 392]
        xts = []
        for et in range(ET):
            tp = psT.tile([128, W2], MMDT, tag="psT")
            srcs = (
                (xa0[:, et * 128 : (et + 1) * 128], 0, L0),
                (xb0[:, et * 128 : (et + 1) * 128], L0, L1),
                (xa1[:, et * 128 : (et + 1) * 128], L, L0),
                (xb1[:, et * 128 : (et + 1) * 128], L + L0, L1),
            )
            for i, (src, off, ln) in enumerate(srcs):
                nc.tensor.matmul(
                    tp[:, off : off + ln], src, ident_r[0:ln, 0:ln],
                    is_transpose=True, start=(i == 0), stop=(i == 3),
                )
            xt_t = xt.tile([128, W2], MMDT, tag="xt")
            nc.any.tensor_copy(out=xt_t, in_=tp)
            xts.append(xt_t)

        # att1^T pair-packed + tanh(+att2 bias)
        tanhs = []
        for at_i in range(AT):
            ap_ps = psA.tile([128, W2], F32, tag="attp")
            for et in range(ET):
                nc.tensor.matmul(
                    ap_ps,
                    we_sb[:, et, at_i * 128 : (at_i + 1) * 128],
                    xts[et][:],
                    start=(et == 0), stop=(et == ET - 1),
                )
            th = tanhp.tile([128, W2], MMDT, tag="tanh")
            for h, b in ((0, b0), (1, b1)):
                nc.scalar.activation(
                    out=th[:, h * L : (h + 1) * L], in_=ap_ps[:, h * L : (h + 1) * L],
                    func=mybir.ActivationFunctionType.Tanh,
                    bias=att2T[:, at_i, b : b + 1], scale=1.0,
                )
            tanhs.append(th)

        # scores [1, 392] at partition 0
        sc_ps = psS.tile([1, W2], F32, tag="scores")
        for at_i in range(AT):
            nc.tensor.matmul(
                sc_ps, wfT[:, at_i : at_i + 1], tanhs[at_i][:],
                start=(at_i == 0), stop=(at_i == AT - 1),
            )

        # masked softmax per 196-segment
        s_sb = small.tile([1, W2], F32, tag="s_sb")
        nc.vector.tensor_add(s_sb, sc_ps, mska)
        nmax = small.tile([1, 2], F32, tag="nmax")
        nc.vector.tensor_reduce(
            out=nmax, in_=s_sb.rearrange("p (b l) -> p b l", b=2),
            axis=mybir.AxisListType.X, op=mybir.AluOpType.max, negate=True,
        )
        e_sb = small.tile([1, W2], F32, tag="e_sb")
        sums = small.tile([1, 2], F32, tag="sums")
        for h in range(2):
            nc.scalar.activation(
                out=e_sb[0:1, h * L : (h + 1) * L],
                in_=s_sb[0:1, h * L : (h + 1) * L],
                func=mybir.ActivationFunctionType.Exp,
                bias=nmax[0:1, h : h + 1], scale=1.0,
                accum_out=sums[0:1, h : h + 1],
            )
        rec = small.tile([1, 2], F32, tag="rec")
        nc.vector.reciprocal(rec, sums)
        alp_t = small.tile([1, W2], F32, tag="alp")
        for h in range(2):
            nc.vector.tensor_scalar_mul(
                alp_t[0:1, h * L : (h + 1) * L],
                e_sb[0:1, h * L : (h + 1) * L],
                rec[0:1, h : h + 1],
            )
        nc.scalar.dma_start(
            out=alp_o[b0 : b0 + 2, :].unsqueeze(0),
            in_=alp_t.rearrange("p (b l) -> p b l", b=2),
        )

        # broadcast alpha across partitions on GpSimd (keeps PE out of the
        # softmax dependency chain entirely)
        bc_sb = scr.tile([128, W2], F32, tag="bcast_sb")
        nc.gpsimd.partition_broadcast(bc_sb, alp_t[0:1, :])

        # awe: fused multiply + accumulate over l on DVE
        for et in range(ET):
            for h, b in ((0, b0), (1, b1)):
                junk = scr.tile([128, L], F32, tag="junk")
                nc.vector.scalar_tensor_tensor(
                    out=junk,
                    in0=xts[et][:, h * L : (h + 1) * L].bitcast(F32),
                    scalar=1.0,
                    in1=bc_sb[:, h * L : (h + 1) * L],
                    op0=mybir.AluOpType.mult,
                    op1=mybir.AluOpType.mult,
                    accum_out=awe_all[:, b * ET + et : b * ET + et + 1],
                )

        if (p2 + 1) % 4 == 0:
            flush_awe_chunk((p2 + 1) // 4 - 1)



_CACHE = {}


def _get_nc():
    if "nc" in _CACHE:
        return _CACHE["nc"]
    nc = bacc.Bacc("TRN2", target_bir_lowering=False, debug=False,
                  num_devices=1)
    aps = {
        "enc": nc.dram_tensor("enc", [BL, L, E], MMDT, kind="ExternalInput").ap(),
        "dec": nc.dram_tensor("dec", [BL, D], F32, kind="ExternalInput").ap(),
        "msk": nc.dram_tensor("msk", [BL, L], I32, kind="ExternalInput").ap(),
        "We": nc.dram_tensor("We", [E, A], MMDT, kind="ExternalInput").ap(),
        "be": nc.dram_tensor("be", [A], MMDT, kind="ExternalInput").ap(),
        "Wd": nc.dram_tensor("Wd", [D, A], MMDT, kind="ExternalInput").ap(),
        "bd": nc.dram_tensor("bd", [A], MMDT, kind="ExternalInput").ap(),
        "wf": nc.dram_tensor("wf", [A], F32, kind="ExternalInput").ap(),
        "awe": nc.dram_tensor("awe", [BL, E], F32, kind="ExternalOutput").ap(),
        "alp": nc.dram_tensor("alp", [BL, L], F32, kind="ExternalOutput").ap(),
    }
    with tile.TileContext(nc) as tc:
        with ExitStack() as ctx:
            build_kernel(ctx, tc, aps)
    nc.compile()
    _CACHE["nc"] = nc
    return nc


def make_in_maps(encoder_out, decoder_hidden, mask1, We, be, Wd, bd, wf):
    enc = np.ascontiguousarray(np.asarray(encoder_out, dtype=np.float32))
    dec = np.ascontiguousarray(np.asarray(decoder_hidden, dtype=np.float32))
    msk = np.ascontiguousarray(np.asarray(mask1, dtype=np.int32))
    wts = {
        "We": np.ascontiguousarray(np.asarray(We, dtype=np.float32)),
        "be": np.ascontiguousarray(np.asarray(be, dtype=np.float32)),
        "Wd": np.ascontiguousarray(np.asarray(Wd, dtype=np.float32)),
        "bd": np.ascontiguousarray(np.asarray(bd, dtype=np.float32)),
        "wf": np.ascontiguousarray(np.asarray(wf, dtype=np.float32)),
    }
    in_maps = []
    for i in range(NCORES):
        sl = slice(i * BL, (i + 1) * BL)
        in_maps.append({
            "enc": enc[sl], "dec": dec[sl], "msk": msk[sl], **wts,
        })
    return in_maps


def kernel(encoder_out, decoder_hidden, mask1, We, be, Wd, bd, wf, bf=None,
           **_ignored):
    from concourse.bass_utils import run_bass_kernel_spmd

    nc = _get_nc()
    in_maps = make_in_maps(encoder_out, decoder_hidden, mask1, We, be, Wd, bd, wf)
    res = run_bass_kernel_spmd(nc, in_maps, core_ids=list(range(NCORES)))
    awe = np.concatenate([r["awe"] for r in res.results], axis=0)
    alp = np.concatenate([r["alp"] for r in res.results], axis=0)
    return awe, alp


if __name__ == "__main__":
    # smoke build
    _get_nc()
    print("build OK")
